# revision 7
# baseline (speedup 1.0000x reference)
"""Dense multi-head attention (S=4096, H=16, D=64) on 8 Trainium2 NeuronCores.

Sharding: heads split across cores (2 heads per core), no cross-core comms.

Host side: Q and K are pre-transposed per head to [D, S] (d-major) so the
kernel DMA-loads Q^T/K^T directly with 16KB-contiguous runs; V stays [S, D].
Q and K are additionally pre-scaled by G = sqrt(1024*log2(e)/sqrt(D)) so the
PSUM logits arrive as x = 1024*log2(e)*s' (s' = scaled logit), which both
exp paths consume directly.

Per-core kernel (per head):
  - DMA K^T/Q^T slices, cast to fp16 into [128, S] tiles (contraction padded
    64->128 for LDWEIGHTS pipelining). Pad row 64 carries Q=1.0 / K=-512.0 so
    every logit lands in PSUM with a -512 offset (turns the DVE exp op's
    round-to-nearest-1024 into floor-to-1024).
  - Load V, cast to fp16 with an appended ones-column -> V' [128, 128]/k-tile.
  - For each 512-wide q chunk, in groups of 3 k-tiles: S^T tiles
    [128 k, 512 q] = KT_tile.T @ QT_chunk, then exp of the 3-bank PSUM group.
    exp alternates between ScalarE (table exp, scale/bias fused) and a custom
    8-block VectorE op (EXP2_FAST_ANT: magic-number floor + quadratic exp2
    mantissa poly, written as int16 bit pattern = fp16 value). Both paths
    produce kappa*exp(s'): the common factor cancels in the softmax divide.
    Splitting exp across two engines removes the ScalarE bottleneck
    (baseline: ACT 256us busy of 304us total).
  - O'^T [128, 512] += V'_tile.T @ E accumulated over all 32 k-tiles.
    Row 64 of O'^T is the softmax denominator (ones-column trick).
  - Epilogue (deferred past the next chunk's first group): PE-transpose
    O'^T back to [128 q, 65], reciprocal of col 64, per-row scale, DMA out.
"""

import numpy as np

import concourse.mybir as mybir
import concourse.tile as tile
from concourse import bacc
from concourse.bass_utils import run_bass_kernel_spmd
from concourse.masks import make_identity

S = 4096
H = 16
D = 64
NCORES = 8
HPC = H // NCORES  # heads per core
NKT = S // 128  # 32 k-tiles per head
NQC = S // 512  # 8 q chunks per head
NCH = NKT // 8  # 4 load chunks per head (1024 columns each)
SCALE = 1.0 / np.sqrt(D)
EXPG = 3  # k-tiles per exp batch (3 psum banks)

F32 = mybir.dt.float32
F16 = mybir.dt.float16
I16 = mybir.dt.int16

# ---- exp formulation constants -------------------------------------------
LOG2E = float(np.log2(np.e))
PRESCALE = float(np.sqrt(1024.0 * LOG2E * SCALE))  # host-applied to q and k
MAGIC = 12884901888.0  # 1.5 * 2^33: fp32 add rounds x to a multiple of 1024
EXP_A = 0.98531599  # quadratic exp2 mantissa poly (minimax incl. borrow)
EXP_B = -3.0719600e-4
EXP_KAPPA = 0.71833097  # common scale of both exp paths; cancels in softmax
MEAN_ADJ = 0.00577  # DVE op's mean rel error, matched in the ACT path
SCALE_ACT = float(np.log(2.0) / 1024.0)
BIAS_ACT = float(np.log(2.0) / 2.0 + np.log(EXP_KAPPA) + MEAN_ADJ)
PAD_Q = 1.0  # pad contraction row 64: q=1, k=-512 -> every logit gets -512
PAD_K = -512.0

# ---- custom DVE op: EXP2_FAST_ANT ----------------------------------------
# out_i16 = tb + P where (all fp32, one 8-block DVE pass, 1 elem/cycle/lane):
#   w  = x + MAGIC          # rounds x to multiple of 1024 (floor, via -512)
#   u  = w - MAGIC          # 1024*k
#   f  = x - u              # mantissa residual in [-512, 512)
#   tb = w - (MAGIC-15360)  # 1024*(15+k): fp16 exponent field
#   P  = f*(EXP_A + f*EXP_B)
# int16(out) bitcast fp16 == kappa * exp(s') to ~1.6% (sawtooth), which the
# softmax normalization reduces to ~<1e-2 of output max.


def _exp_reference(in0, in1, s0, s1, imm2):
    x = in0.astype(np.float32)
    w = (x + np.float32(s0)).astype(np.float32)
    u = (w - np.float32(s0)).astype(np.float32)
    f = (x - u).astype(np.float32)
    tb = (w - np.float32(s1)).astype(np.float32)
    return (tb + f * (np.float32(imm2) + f * in1)).astype(np.float32)


def _register_exp_op():
    import concourse.dve_ops as dops
    from concourse.dve_spec import (
        C0,
        C1,
        C2,
        C3,
        Spec,
        Src0,
        _has_src1,
        _spill_c3_to_src1,
        lower,
    )
    from concourse.dve_uop import DveOpSpec

    name = "EXP2_FAST_ANT"
    for op in dops.OPS:
        if op.name == name:
            return op
    w = Src0 + C0
    u = w - C0
    f = Src0 - u
    tb = w - C1
    P = (f * C3 + C2) * f
    body = _spill_c3_to_src1(tb + P)
    spec = Spec(body=body, reference=_exp_reference)
    row = max(dops._SUB_OPCODE_FOR_NAME.values()) + 1
    assert row < 0x20
    dops._SUB_OPCODE_FOR_NAME[name] = row
    shas = {}
    for ver in ("v3", "v4"):
        tmp = DveOpSpec(
            name=name, opcode=row, uops=lower(spec, ver=ver), rd1_en=_has_src1(spec)
        )
        shas[ver] = tmp.sha(ver)
    op = dops.DveOp(name, spec, subdim=False, uops_sha=shas)
    dops.OPS.append(op)
    dops.CUSTOM_DVE_SPECS[name] = spec
    return op


EXP_OP = _register_exp_op()


def _groups():
    """Split NKT k-tiles into exp groups of EXPG (last group smaller)."""
    out = []
    t = 0
    while t < NKT:
        g = min(EXPG, NKT - t)
        out.append((t, g))
        t += g
    return out


def _build_head(nc, tc, pools, idn16, bco, bact, q, k, v, o, h):
    sb, epool, spsum, opsum = pools

    # ---- Phase A: load K^T/Q^T slices + V, cast everything to fp16 ----
    # qt/kt hold Q^T/K^T on partitions 0..63; partition 64 carries the
    # -512 pad pair; partitions 65..127 are zero.
    qts = [
        sb.tile([128, 1024], F16, tag=f"qt{b}", name=f"qt{b}") for b in range(NCH)
    ]
    kts = [
        sb.tile([128, 1024], F16, tag=f"kt{b}", name=f"kt{b}") for b in range(NCH)
    ]
    nc.gpsimd.memset(kts[0][D:128, :], 0.0)
    nc.gpsimd.memset(qts[0][D:128, :], 0.0)
    nc.gpsimd.memset(kts[0][D : D + 1, :], PAD_K)
    nc.gpsimd.memset(qts[0][D : D + 1, :], PAD_Q)

    def qk_chunk(src, dsts, b):
        stg = sb.tile([D, 1024], F32, tag="stg", bufs=3)
        nc.sync.dma_start(stg[:], src.ap()[h, :, b * 1024 : (b + 1) * 1024])
        nc.gpsimd.tensor_copy(dsts[b][0:D, :], stg[:])

    qk_chunk(k, kts, 0)
    qk_chunk(q, qts, 0)
    for t_ in qts[1:]:
        nc.gpsimd.memset(t_[D:128, :], 0.0)
        nc.gpsimd.memset(t_[D : D + 1, :], PAD_Q)
    for t_ in kts[1:]:
        nc.gpsimd.memset(t_[D:128, :], 0.0)
        nc.gpsimd.memset(t_[D : D + 1, :], PAD_K)
    qk_chunk(k, kts, 1)
    qk_chunk(q, qts, 1)

    # V' padded to 128 columns so the PV LDWEIGHTS gets fast-weight-load:
    # col D is the ones column (softmax denominator), cols D+1.. are zero.
    vst32 = sb.tile([128, NKT, D], F32, tag="vst32")
    nc.sync.dma_start(vst32[:], v.ap()[h].rearrange("(n p) d -> p n d", p=128))
    vstage = sb.tile([128, NKT, 128], F16, tag="vstage")
    nc.gpsimd.memset(vstage[:, :, D + 1 : 128], 0.0)
    nc.gpsimd.tensor_copy(vstage[:, :, 0:D], vst32[:])
    ones = sb.tile([128, NKT], F32, tag="ones")
    nc.gpsimd.memset(ones[:], 1.0)
    nc.gpsimd.tensor_copy(vstage[:, :, D], ones[:])

    for b in range(2, NCH):
        qk_chunk(k, kts, b)
        qk_chunk(q, qts, b)

    # ---- Phase B: attention, software-pipelined one exp-group deep ----
    def qk_group(qc, t0, glen):
        qs = qc * 512
        sp = spsum.tile([128, EXPG * 512], F32, tag="sp")
        for j in range(glen):
            t = t0 + j
            nc.tensor.matmul(
                sp[:, j * 512 : (j + 1) * 512],
                kts[t // 8][:, (t % 8) * 128 : (t % 8 + 1) * 128],
                qts[qc // 2][:, (qs % 1024) : (qs % 1024) + 512],
            )
        return sp

    def epilogue(ot, qs):
        tp2 = opsum.tile([128, 512], F16, tag="acc")
        for j in range(4):
            nc.tensor.matmul(
                tp2[:, j * 128 : j * 128 + D + 1],
                ot[:, j * 128 : (j + 1) * 128],
                idn16[0 : D + 1, 0 : D + 1],
                is_transpose=True,
            )
        otT = sb.tile([128, 512], F16, tag="otT")
        nc.vector.tensor_copy(otT[:], tp2[:])
        fin = sb.tile([128, 4, D], F32, tag="fin")
        rcp = sb.tile([128, 4], F32, tag="rcp")
        nc.vector.reciprocal(
            rcp[:], otT[:].rearrange("p (j c) -> p j c", c=128)[:, :, D]
        )
        for j in range(4):
            nc.vector.tensor_scalar_mul(
                fin[:, j, :],
                otT[:, j * 128 : j * 128 + D],
                rcp[:, j : j + 1],
            )
        nc.sync.dma_start(
            o.ap()[h, qs : qs + 512, :].rearrange("(n p) d -> p n d", p=128),
            fin[:],
        )

    groups = [
        (gi, qc, t0, glen)
        for qc in range(NQC)
        for gi, (t0, glen) in enumerate(_groups())
    ]
    sp_next = qk_group(*groups[0][1:])
    acc = None
    pending = None
    for i, (gi, qc, t0, glen) in enumerate(groups):
        sp = sp_next
        et = epool.tile([128, EXPG * 512], F16, tag="et")
        if gi % 2 == 1:
            # VectorE path: custom 8-block exp, int16 bit pattern = fp16
            nc.vector._custom_dve(
                EXP_OP,
                out=et[:, 0 : glen * 512].bitcast(I16),
                in0=sp[:, 0 : glen * 512],
                in1=bco[:, 0:1],
                s0=MAGIC,
                s1=MAGIC - 15360.0,
                imm2=EXP_A,
            )
        else:
            nc.scalar.activation(
                et[:, 0 : glen * 512],
                sp[:, 0 : glen * 512],
                mybir.ActivationFunctionType.Exp,
                scale=SCALE_ACT,
                bias=bact[:, 0:1],
            )
        if i + 1 < len(groups):
            sp_next = qk_group(*groups[i + 1][1:])
        if t0 == 0:
            if pending is not None:
                epilogue(*pending)
                pending = None
            acc = opsum.tile([128, 512], F32, tag="acc")
        for j in range(glen):
            t = t0 + j
            nc.tensor.matmul(
                acc[:],
                vstage[:, t, :],
                et[:, j * 512 : (j + 1) * 512],
                start=(t == 0),
                stop=(t == NKT - 1),
            )
        if t0 + glen == NKT:
            # eager: copy the accumulator out (fp16) so its PSUM slot frees
            ot = sb.tile([D + 1, 512], F16, tag="ot")
            nc.vector.tensor_copy(ot[:], acc[0 : D + 1, :])
            pending = (ot, qc * 512)
    epilogue(*pending)


def _build():
    nc = bacc.Bacc(trn_type="TRN2", debug=False, num_devices=NCORES)
    q = nc.dram_tensor("q", [HPC, D, S], F32, kind="ExternalInput")
    k = nc.dram_tensor("k", [HPC, D, S], F32, kind="ExternalInput")
    v = nc.dram_tensor("v", [HPC, S, D], F32, kind="ExternalInput")
    o = nc.dram_tensor("o", [HPC, S, D], F32, kind="ExternalOutput")

    with tile.TileContext(nc) as tc:
        with (
            tc.tile_pool(name="const", bufs=1) as cpool,
            tc.tile_pool(name="sb", bufs=2) as sb,
            tc.tile_pool(name="epool", bufs=3) as epool,
            tc.tile_pool(name="spsum", bufs=2, space="PSUM") as spsum,
            tc.tile_pool(name="opsum", bufs=2, space="PSUM") as opsum,
        ):
            # Dummy exp at t~0 pulls the ACT table-load DMA in front of the
            # input DMAs (otherwise the first input chunk queues behind it).
            warm = cpool.tile([128, 1], F32, tag="warm")
            nc.gpsimd.memset(warm[:], 0.0)
            nc.scalar.activation(
                warm[:], warm[:], mybir.ActivationFunctionType.Exp
            )
            idn = cpool.tile([128, 128], F32, tag="idn")
            make_identity(nc, idn[:])
            idn16 = cpool.tile([128, 128], F16, tag="idn16")
            nc.vector.tensor_copy(idn16[:], idn[:])
            bco = cpool.tile([128, 1], F32, tag="bco")
            nc.gpsimd.memset(bco[:], EXP_B)
            bact = cpool.tile([128, 1], F32, tag="bact")
            nc.gpsimd.memset(bact[:], BIAS_ACT)
            pools = (sb, epool, spsum, opsum)
            for h in range(HPC):
                _build_head(nc, tc, pools, idn16, bco, bact, q, k, v, o, h)

    nc.compile()
    return nc


_NC_CACHE = None


def prepare_in_maps(query, key, value):
    """Host prep: per-core slices, prescale q/k, pre-transpose to [H, D, S]."""
    query = np.asarray(query)
    key = np.asarray(key)
    value = np.asarray(value)
    g = np.float32(PRESCALE)
    in_maps = []
    for c in range(NCORES):
        sl = slice(c * HPC, (c + 1) * HPC)
        in_maps.append(
            {
                # [S, HPC, D] -> [HPC, D, S] (pre-transposed Q^T/K^T)
                "q": np.ascontiguousarray(
                    (query[:, sl, :] * g).transpose(1, 2, 0)
                ),
                "k": np.ascontiguousarray(
                    (key[:, sl, :] * g).transpose(1, 2, 0)
                ),
                # [S, HPC, D] -> [HPC, S, D]
                "v": np.ascontiguousarray(value[:, sl, :].transpose(1, 0, 2)),
            }
        )
    return in_maps


def kernel(query, key, value):
    global _NC_CACHE
    if _NC_CACHE is None:
        _NC_CACHE = _build()
    nc = _NC_CACHE

    in_maps = prepare_in_maps(query, key, value)
    res = run_bass_kernel_spmd(nc, in_maps, core_ids=list(range(NCORES)))
    out = np.concatenate(
        [res.results[c]["o"].transpose(1, 0, 2) for c in range(NCORES)], axis=1
    )
    return out


# revision 11
# speedup vs baseline: 1.2314x; 1.2314x over previous
"""Dense multi-head attention (S=4096, H=16, D=64) on 8 Trainium2 NeuronCores.

Sharding: heads split across cores (2 heads per core), no cross-core comms.

Host side: Q and K are pre-transposed per head to [D, S] (d-major) so the
kernel DMA-loads Q^T/K^T directly with 16KB-contiguous runs; V stays [S, D].
Q and K are additionally pre-scaled by G = sqrt(1024*log2(e)/sqrt(D)) so the
PSUM logits arrive as x = 1024*log2(e)*s' (s' = scaled logit), which both
exp paths consume directly.

Per-core kernel (per head):
  - DMA K^T/Q^T slices, cast to fp16 into [128, S] tiles (contraction padded
    64->128 for LDWEIGHTS pipelining). Pad row 64 carries Q=1.0 / K=-512.0 so
    every logit lands in PSUM with a -512 offset (turns the DVE exp op's
    round-to-nearest-1024 into floor-to-1024).
  - Load V, cast to fp16 with an appended ones-column -> V' [128, 128]/k-tile.
  - For each 512-wide q chunk, in groups of 3 k-tiles: S^T tiles
    [128 k, 512 q] = KT_tile.T @ QT_chunk, then exp of the 3-bank PSUM group.
    exp alternates between ScalarE (table exp, scale/bias fused) and a custom
    8-block VectorE op (EXP2_FAST_ANT: magic-number floor + quadratic exp2
    mantissa poly, written as int16 bit pattern = fp16 value). Both paths
    produce kappa*exp(s'): the common factor cancels in the softmax divide.
    Splitting exp across two engines removes the ScalarE bottleneck
    (baseline: ACT 256us busy of 304us total).
  - O'^T [128, 512] += V'_tile.T @ E accumulated over all 32 k-tiles.
    Row 64 of O'^T is the softmax denominator (ones-column trick).
  - Epilogue (deferred past the next chunk's first group): PE-transpose
    O'^T back to [128 q, 65], reciprocal of col 64, per-row scale, DMA out.
"""

import numpy as np

import concourse.mybir as mybir
import concourse.tile as tile
from concourse import bacc
from concourse.bass_utils import run_bass_kernel_spmd
from concourse.masks import make_identity

S = 4096
H = 16
D = 64
NCORES = 8
HPC = H // NCORES  # heads per core
NKT = S // 128  # 32 k-tiles per head
NQC = S // 512  # 8 q chunks per head
NCH = NKT // 8  # 4 load chunks per head (1024 columns each)
SCALE = 1.0 / np.sqrt(D)
EXPG = 3  # k-tiles per exp batch (3 psum banks)

F32 = mybir.dt.float32
F16 = mybir.dt.float16
I16 = mybir.dt.int16

# ---- exp formulation constants -------------------------------------------
LOG2E = float(np.log2(np.e))
PRESCALE = float(np.sqrt(1024.0 * LOG2E * SCALE))  # host-applied to q and k
MAGIC = 12884901888.0  # 1.5 * 2^33: fp32 add rounds x to a multiple of 1024
EXP_A = 0.98531599  # quadratic exp2 mantissa poly (minimax incl. borrow)
EXP_B = -3.0719600e-4
EXP_KAPPA = 0.71833097  # common scale of both exp paths; cancels in softmax
MEAN_ADJ = 0.00577  # DVE op's mean rel error, matched in the ACT path
SCALE_ACT = float(np.log(2.0) / 1024.0)
BIAS_ACT = float(np.log(2.0) / 2.0 + np.log(EXP_KAPPA) + MEAN_ADJ)
PAD_Q = 1.0  # pad contraction row 64: q=1, k=-512 -> every logit gets -512
PAD_K = -512.0

# ---- custom DVE op: EXP2_FAST_ANT ----------------------------------------
# out_i16 = tb + P where (all fp32, one 8-block DVE pass, 1 elem/cycle/lane):
#   w  = x + MAGIC          # rounds x to multiple of 1024 (floor, via -512)
#   u  = w - MAGIC          # 1024*k
#   f  = x - u              # mantissa residual in [-512, 512)
#   tb = w - (MAGIC-15360)  # 1024*(15+k): fp16 exponent field
#   P  = f*(EXP_A + f*EXP_B)
# int16(out) bitcast fp16 == kappa * exp(s') to ~1.6% (sawtooth), which the
# softmax normalization reduces to ~<1e-2 of output max.


def _exp_reference(in0, in1, s0, s1, imm2):
    x = in0.astype(np.float32)
    w = (x + np.float32(s0)).astype(np.float32)
    u = (w - np.float32(s0)).astype(np.float32)
    f = (x - u).astype(np.float32)
    tb = (w - np.float32(s1)).astype(np.float32)
    return (tb + f * (np.float32(imm2) + f * in1)).astype(np.float32)


def _register_exp_op():
    import concourse.dve_ops as dops
    from concourse.dve_spec import (
        C0,
        C1,
        C2,
        C3,
        Spec,
        Src0,
        _has_src1,
        _spill_c3_to_src1,
        lower,
    )
    from concourse.dve_uop import DveOpSpec

    name = "EXP2_FAST_ANT"
    for op in dops.OPS:
        if op.name == name:
            return op
    w = Src0 + C0
    u = w - C0
    f = Src0 - u
    tb = w - C1
    P = (f * C3 + C2) * f
    body = _spill_c3_to_src1(tb + P)
    spec = Spec(body=body, reference=_exp_reference)
    row = max(dops._SUB_OPCODE_FOR_NAME.values()) + 1
    assert row < 0x20
    dops._SUB_OPCODE_FOR_NAME[name] = row
    shas = {}
    for ver in ("v3", "v4"):
        tmp = DveOpSpec(
            name=name, opcode=row, uops=lower(spec, ver=ver), rd1_en=_has_src1(spec)
        )
        shas[ver] = tmp.sha(ver)
    op = dops.DveOp(name, spec, subdim=False, uops_sha=shas)
    dops.OPS.append(op)
    dops.CUSTOM_DVE_SPECS[name] = spec
    return op


EXP_OP = _register_exp_op()


def _groups():
    """Split NKT k-tiles into exp groups of EXPG (last group smaller)."""
    out = []
    t = 0
    while t < NKT:
        g = min(EXPG, NKT - t)
        out.append((t, g))
        t += g
    return out


def _build_head(nc, tc, pools, idn16, bco, bact, q, k, v, o, h):
    sb, epool, spsum, opsum = pools

    # ---- Phase A: load K^T/Q^T slices + V, cast everything to fp16 ----
    # qt/kt hold Q^T/K^T on partitions 0..63; partition 64 carries the
    # -512 pad pair; partitions 65..127 are zero.
    qts = [
        sb.tile([128, 1024], F16, tag=f"qt{b}", name=f"qt{b}") for b in range(NCH)
    ]
    kts = [
        sb.tile([128, 1024], F16, tag=f"kt{b}", name=f"kt{b}") for b in range(NCH)
    ]
    nc.gpsimd.memset(kts[0][D:128, :], 0.0)
    nc.gpsimd.memset(qts[0][D:128, :], 0.0)
    nc.gpsimd.memset(kts[0][D : D + 1, :], PAD_K)
    nc.gpsimd.memset(qts[0][D : D + 1, :], PAD_Q)

    def qk_chunk(src, dsts, b):
        stg = sb.tile([D, 1024], F32, tag="stg", bufs=3)
        nc.sync.dma_start(stg[:], src.ap()[h, :, b * 1024 : (b + 1) * 1024])
        nc.vector.tensor_copy(dsts[b][0:D, :], stg[:])

    qk_chunk(k, kts, 0)
    qk_chunk(q, qts, 0)
    for t_ in qts[1:]:
        nc.gpsimd.memset(t_[D:128, :], 0.0)
        nc.gpsimd.memset(t_[D : D + 1, :], PAD_Q)
    for t_ in kts[1:]:
        nc.gpsimd.memset(t_[D:128, :], 0.0)
        nc.gpsimd.memset(t_[D : D + 1, :], PAD_K)
    qk_chunk(k, kts, 1)
    qk_chunk(q, qts, 1)

    # V' padded to 128 columns so the PV LDWEIGHTS gets fast-weight-load:
    # col D is the ones column (softmax denominator), cols D+1.. are zero.
    vst32 = sb.tile([128, NKT, D], F32, tag="vst32")
    nc.sync.dma_start(vst32[:], v.ap()[h].rearrange("(n p) d -> p n d", p=128))
    vstage = sb.tile([128, NKT, 128], F16, tag="vstage")
    nc.gpsimd.memset(vstage[:, :, D + 1 : 128], 0.0)
    nc.vector.tensor_copy(vstage[:, :, 0:D], vst32[:])
    ones = sb.tile([128, NKT], F32, tag="ones")
    nc.gpsimd.memset(ones[:], 1.0)
    nc.vector.tensor_copy(vstage[:, :, D], ones[:])

    for b in range(2, NCH):
        qk_chunk(k, kts, b)
        qk_chunk(q, qts, b)

    # ---- Phase B: attention, software-pipelined one exp-group deep ----
    def qk_group(qc, t0, glen):
        qs = qc * 512
        sp = spsum.tile([128, EXPG * 512], F32, tag="sp")
        for j in range(glen):
            t = t0 + j
            nc.tensor.matmul(
                sp[:, j * 512 : (j + 1) * 512],
                kts[t // 8][:, (t % 8) * 128 : (t % 8 + 1) * 128],
                qts[qc // 2][:, (qs % 1024) : (qs % 1024) + 512],
            )
        return sp

    def epilogue(ot, qs):
        tp2 = opsum.tile([128, 512], F16, tag="acc")
        for j in range(4):
            nc.tensor.matmul(
                tp2[:, j * 128 : j * 128 + D + 1],
                ot[:, j * 128 : (j + 1) * 128],
                idn16[0 : D + 1, 0 : D + 1],
                is_transpose=True,
            )
        otT = sb.tile([128, 512], F16, tag="otT")
        nc.vector.tensor_copy(otT[:], tp2[:])
        fin = sb.tile([128, 4, D], F32, tag="fin")
        rcp = sb.tile([128, 4], F32, tag="rcp")
        nc.vector.reciprocal(
            rcp[:], otT[:].rearrange("p (j c) -> p j c", c=128)[:, :, D]
        )
        for j in range(4):
            nc.vector.tensor_scalar_mul(
                fin[:, j, :],
                otT[:, j * 128 : j * 128 + D],
                rcp[:, j : j + 1],
            )
        nc.sync.dma_start(
            o.ap()[h, qs : qs + 512, :].rearrange("(n p) d -> p n d", p=128),
            fin[:],
        )

    groups = [
        (gi, qc, t0, glen)
        for qc in range(NQC)
        for gi, (t0, glen) in enumerate(_groups())
    ]
    n = len(groups)
    ets = {}
    state = {"acc": None, "pending": None}

    def issue_exp(i, sp):
        gi, qc, t0, glen = groups[i]
        et = epool.tile([128, EXPG * 512], F16, tag="et")
        if gi % 2 == 1:
            # VectorE path: custom 8-block exp, int16 bit pattern = fp16
            nc.vector._custom_dve(
                EXP_OP,
                out=et[:, 0 : glen * 512].bitcast(I16),
                in0=sp[:, 0 : glen * 512],
                in1=bco[:, 0:1],
                s0=MAGIC,
                s1=MAGIC - 15360.0,
                imm2=EXP_A,
            )
        else:
            nc.scalar.activation(
                et[:, 0 : glen * 512],
                sp[:, 0 : glen * 512],
                mybir.ActivationFunctionType.Exp,
                scale=SCALE_ACT,
                bias=bact[:, 0:1],
            )
        ets[i] = et

    def issue_pv(i):
        gi, qc, t0, glen = groups[i]
        et = ets.pop(i)
        if t0 == 0:
            if state["pending"] is not None:
                epilogue(*state["pending"])
                state["pending"] = None
            state["acc"] = opsum.tile([128, 512], F32, tag="acc", name="acc")
        acc = state["acc"]
        for j in range(glen):
            t = t0 + j
            nc.tensor.matmul(
                acc[:],
                vstage[:, t, :],
                et[:, j * 512 : (j + 1) * 512],
                start=(t == 0),
                stop=(t == NKT - 1),
            )
        if t0 + glen == NKT:
            # eager: copy the accumulator out (fp16) so its PSUM slot frees
            ot = sb.tile([D + 1, 512], F16, tag="ot")
            nc.vector.tensor_copy(ot[:], acc[0 : D + 1, :])
            state["pending"] = (ot, qc * 512)

    # PV lags exp by one group so the in-order PE stream (QK(i+1), PV(i-1))
    # never waits on an exp still in flight on ScalarE/VectorE.
    sp_next = qk_group(*groups[0][1:])
    for i in range(n):
        issue_exp(i, sp_next)
        if i + 1 < n:
            sp_next = qk_group(*groups[i + 1][1:])
        if i >= 1:
            issue_pv(i - 1)
    issue_pv(n - 1)
    epilogue(*state["pending"])


def _build():
    nc = bacc.Bacc(trn_type="TRN2", debug=False, num_devices=NCORES)
    q = nc.dram_tensor("q", [HPC, D, S], F32, kind="ExternalInput")
    k = nc.dram_tensor("k", [HPC, D, S], F32, kind="ExternalInput")
    v = nc.dram_tensor("v", [HPC, S, D], F32, kind="ExternalInput")
    o = nc.dram_tensor("o", [HPC, S, D], F32, kind="ExternalOutput")

    with tile.TileContext(nc) as tc:
        with (
            tc.tile_pool(name="const", bufs=1) as cpool,
            tc.tile_pool(name="sb", bufs=2) as sb,
            tc.tile_pool(name="epool", bufs=3) as epool,
            tc.tile_pool(name="spsum", bufs=2, space="PSUM") as spsum,
            tc.tile_pool(name="opsum", bufs=2, space="PSUM") as opsum,
        ):
            # Dummy exp at t~0 pulls the ACT table-load DMA in front of the
            # input DMAs (otherwise the first input chunk queues behind it).
            warm = cpool.tile([128, 1], F32, tag="warm")
            nc.gpsimd.memset(warm[:], 0.0)
            nc.scalar.activation(
                warm[:], warm[:], mybir.ActivationFunctionType.Exp
            )
            idn = cpool.tile([128, 128], F32, tag="idn")
            make_identity(nc, idn[:])
            idn16 = cpool.tile([128, 128], F16, tag="idn16")
            nc.vector.tensor_copy(idn16[:], idn[:])
            bco = cpool.tile([128, 1], F32, tag="bco")
            nc.gpsimd.memset(bco[:], EXP_B)
            bact = cpool.tile([128, 1], F32, tag="bact")
            nc.gpsimd.memset(bact[:], BIAS_ACT)
            pools = (sb, epool, spsum, opsum)
            for h in range(HPC):
                _build_head(nc, tc, pools, idn16, bco, bact, q, k, v, o, h)

    nc.compile()
    return nc


_NC_CACHE = None


def prepare_in_maps(query, key, value):
    """Host prep: per-core slices, prescale q/k, pre-transpose to [H, D, S]."""
    query = np.asarray(query)
    key = np.asarray(key)
    value = np.asarray(value)
    g = np.float32(PRESCALE)
    in_maps = []
    for c in range(NCORES):
        sl = slice(c * HPC, (c + 1) * HPC)
        in_maps.append(
            {
                # [S, HPC, D] -> [HPC, D, S] (pre-transposed Q^T/K^T)
                "q": np.ascontiguousarray(
                    (query[:, sl, :] * g).transpose(1, 2, 0)
                ),
                "k": np.ascontiguousarray(
                    (key[:, sl, :] * g).transpose(1, 2, 0)
                ),
                # [S, HPC, D] -> [HPC, S, D]
                "v": np.ascontiguousarray(value[:, sl, :].transpose(1, 0, 2)),
            }
        )
    return in_maps


def kernel(query, key, value):
    global _NC_CACHE
    if _NC_CACHE is None:
        _NC_CACHE = _build()
    nc = _NC_CACHE

    in_maps = prepare_in_maps(query, key, value)
    res = run_bass_kernel_spmd(nc, in_maps, core_ids=list(range(NCORES)))
    out = np.concatenate(
        [res.results[c]["o"].transpose(1, 0, 2) for c in range(NCORES)], axis=1
    )
    return out
